# revision 6
# baseline (speedup 1.0000x reference)
import contextlib
import os
import sys

os.environ.setdefault("MYCRO_LOCAL_CACHE", "1")
for _p in ("/opt/trn_rl_repo",):
    if os.path.isdir(_p) and _p not in sys.path:
        sys.path.append(_p)

import ml_dtypes
import numpy as np

import concourse.bass as bass
from concourse import bacc
import concourse.mybir as mybir
import concourse.tile as tile
from concourse.bass_utils import run_bass_kernel_spmd

FP = mybir.dt.float32
FPR = mybir.dt.float32r
BF = mybir.dt.bfloat16
AF = mybir.ActivationFunctionType

B, N, D, H = 2, 2048, 1024, 16
NCORES = 8
GRP = 4
HPC = H // GRP
C = D // GRP
R = N // GRP
DH = D // H
SCALE = DH ** -0.5
LN_EPS = 1e-5

NT = N // 128
KD = D // 128
NS = N // 512

ATT_DT = BF
PROJ_DT = BF


def build():
    nc = bacc.Bacc("TRN2", target_bir_lowering=False, num_devices=NCORES)

    xT_t = nc.dram_tensor("xT", [D, N], PROJ_DT, kind="ExternalInput")
    posT_t = nc.dram_tensor("posT", [C, N], FP, kind="ExternalInput")
    wq_t = nc.dram_tensor("wq", [D, C], PROJ_DT, kind="ExternalInput")
    wk_t = nc.dram_tensor("wk", [D, C], PROJ_DT, kind="ExternalInput")
    wv_t = nc.dram_tensor("wv", [D, C], PROJ_DT, kind="ExternalInput")
    wo_t = nc.dram_tensor("wo", [C, D], BF, kind="ExternalInput")
    res_t = nc.dram_tensor("resid", [R, D], FP, kind="ExternalInput")
    g_t = nc.dram_tensor("ln_g", [D], FP, kind="ExternalInput")
    bt_t = nc.dram_tensor("ln_b", [D], FP, kind="ExternalInput")
    out_t = nc.dram_tensor("out", [R, D], FP, kind="ExternalOutput")

    res_tiles = res_t.ap().rearrange("(t p) d -> t p d", p=128)
    out_tiles = out_t.ap().rearrange("(t p) d -> t p d", p=128)

    def bcast_ap(ap, parts):
        return bass.AP(tensor=ap.tensor, offset=ap.offset,
                       ap=[[0, parts]] + list(ap.ap))

    with tile.TileContext(nc) as tc, contextlib.ExitStack() as ctx:
        persist = ctx.enter_context(tc.tile_pool(name="persist", bufs=1))
        attnp = ctx.enter_context(tc.tile_pool(name="attnp", bufs=1))
        psP = ctx.enter_context(tc.tile_pool(name="psP", bufs=1, space="PSUM"))
        psO = ctx.enter_context(tc.tile_pool(name="psO", bufs=3, space="PSUM"))
        psC = ctx.enter_context(tc.tile_pool(name="psC", bufs=2, space="PSUM"))
        dram = ctx.enter_context(tc.tile_pool(name="dram", bufs=1, space="DRAM"))

        ones64 = persist.tile([1, DH], FP, tag="ones64")
        nc.vector.memset(ones64, 1.0)
        onescol = persist.tile([128, 1], FP, tag="onescol")
        nc.vector.memset(onescol, 1.0)

        sbA = ctx.enter_context(tc.tile_pool(name="sbA", bufs=3))

        ph12_ctx = contextlib.ExitStack()
        p12 = ph12_ctx.enter_context(tc.tile_pool(name="ph12", bufs=1))

        wq_sb = p12.tile([128, KD, C], PROJ_DT, tag="wq")
        wk_sb = p12.tile([128, KD, C], PROJ_DT, tag="wk")
        wv_sb = p12.tile([128, KD, C], PROJ_DT, tag="wv")
        xT_sb = p12.tile([128, KD, N], PROJ_DT, tag="xT")
        xT_src = xT_t.ap().rearrange("(k p) n -> p k n", p=128)
        nc.sync.dma_start(out=wq_sb, in_=wq_t.ap().rearrange("(k p) c -> p k c", p=128))
        for k in range(KD):
            nc.sync.dma_start(out=xT_sb[:, k, :], in_=xT_src[:, k, :])
        nc.sync.dma_start(out=wk_sb, in_=wk_t.ap().rearrange("(k p) c -> p k c", p=128))
        nc.sync.dma_start(out=wv_sb, in_=wv_t.ap().rearrange("(k p) c -> p k c", p=128))
        xT = [xT_sb[:, k, :] for k in range(KD)]

        posT_sb = p12.tile([128, 2, N], FP, tag="posT")
        nc.sync.dma_start(out=posT_sb,
                          in_=posT_t.ap().rearrange("(m p) n -> p m n", p=128))
        posT = [posT_sb[:, m, :] for m in range(2)]

        qT = [attnp.tile([128, N], ATT_DT, name=f"qT{m}", tag=f"qT{m}") for m in range(2)]
        kpT = [attnp.tile([128, N], ATT_DT, name=f"kpT{m}", tag=f"kpT{m}") for m in range(2)]
        V = [attnp.tile([128, HPC, DH + 1], ATT_DT, name=f"V{t}", tag=f"V{t}")
             for t in range(NT)]

        def proj_qkp(m):
            for s in range(NS):
                q_ps = psP.tile([128, 512], FP, tag="ps", name="q_ps")
                for k in range(KD):
                    nc.tensor.matmul(q_ps, wq_sb[:, k, m * 128:(m + 1) * 128],
                                     xT[k][:, s * 512:(s + 1) * 512],
                                     start=(k == 0), stop=(k == KD - 1))
                nc.vector.tensor_copy(out=qT[m][:, s * 512:(s + 1) * 512], in_=q_ps)
            for s in range(NS):
                kp_ps = psP.tile([128, 512], FP, tag="ps", name="kp_ps")
                for k in range(KD):
                    nc.tensor.matmul(kp_ps, wk_sb[:, k, m * 128:(m + 1) * 128],
                                     xT[k][:, s * 512:(s + 1) * 512],
                                     start=(k == 0), stop=(k == KD - 1))
                nc.vector.tensor_add(out=kpT[m][:, s * 512:(s + 1) * 512],
                                     in0=kp_ps, in1=posT[m][:, s * 512:(s + 1) * 512])

        proj_qkp(0)
        for t in range(NT):
            v_ps = psP.tile([128, C], FP, tag="ps", name="v_ps")
            for k in range(KD):
                nc.tensor.matmul(v_ps, xT[k][:, t * 128:(t + 1) * 128], wv_sb[:, k, :],
                                 start=(k == 0), stop=(k == KD - 1))
            nc.vector.tensor_copy(out=V[t][:, :, 0:DH],
                                  in_=v_ps.rearrange("p (h d) -> p h d", h=HPC))
            nc.vector.tensor_copy(out=V[t][:, :, DH:DH + 1],
                                  in_=onescol.broadcast_to([128, HPC, 1]))

        pools = {}

        wo_sb = persist.tile([128, 2, D], BF, tag="wo")
        nc.sync.dma_start(out=wo_sb, in_=wo_t.ap().rearrange("(k p) d -> p k d", p=128))
        g_sb = persist.tile([128, D], FP, tag="g")
        b_sb = persist.tile([128, D], FP, tag="b")
        nc.gpsimd.dma_start(out=g_sb, in_=bcast_ap(g_t.ap(), 128))
        nc.gpsimd.dma_start(out=b_sb, in_=bcast_ap(bt_t.ap(), 128))
        eps_sb = persist.tile([128, 1], FP, tag="eps")
        nc.vector.memset(eps_sb, LN_EPS)

        OT = [attnp.tile([128, N], BF, name=f"OT{m}", tag=f"OT{m}") for m in range(2)]
        OTU = [attnp.tile([128, N], FP, name=f"OTU{m}", tag=f"OTU{m}") for m in range(2)]
        oph = [dram.tile([R, D], BF, name=f"oph{s}", tag=f"oph{s}") for s in range(NS)]
        rsh = [dram.tile([128, D], BF, name=f"rsh{s}", tag=f"rsh{s}") for s in range(NS)]

        def attention(s, hp):
            ot_e = psO.tile([128, 512], FP, tag="ot", name="ot_e")
            ot_o = psO.tile([128, 512], FP, tag="ot", name="ot_o")
            for jt in range(NT):
                st = psC.tile([128, 1024], FP, tag="st", name="st")
                nc.tensor.matmul(st[:, 0:512],
                                 kpT[hp][0:64, jt * 128:(jt + 1) * 128],
                                 qT[hp][0:64, s * 512:(s + 1) * 512],
                                 start=True, stop=True)
                nc.tensor.matmul(st[:, 512:1024],
                                 kpT[hp][64:128, jt * 128:(jt + 1) * 128],
                                 qT[hp][64:128, s * 512:(s + 1) * 512],
                                 start=True, stop=True)
                ste = sbA.tile([128, 1024], ATT_DT, tag="ste", name="ste")
                nc.scalar.activation(out=ste, in_=st, func=AF.Exp, scale=SCALE)
                nc.tensor.matmul(ot_e[0:DH + 1, :], V[jt][:, 2 * hp, :],
                                 ste[:, 0:512],
                                 start=(jt == 0), stop=(jt == NT - 1))
                nc.tensor.matmul(ot_o[0:DH + 1, :], V[jt][:, 2 * hp + 1, :],
                                 ste[:, 512:1024],
                                 start=(jt == 0), stop=(jt == NT - 1))
            jobs = []
            for par, ot in ((0, ot_e), (1, ot_o)):
                csrow = sbA.tile([1, 512], FP, tag="csrow", name="csrow", bufs=8)
                nc.vector.tensor_copy(out=csrow, in_=ot[DH:DH + 1, :])
                dst = OT[hp][par * 64:par * 64 + DH, s * 512:(s + 1) * 512]
                dstu = OTU[hp][par * 64:par * 64 + DH, s * 512:(s + 1) * 512]
                nc.vector.tensor_copy(out=dstu, in_=ot[0:DH, :])
                jobs.append((dst, dstu, csrow, par))
            return jobs

        def normalize(jobs):
            for dst, dstu, csrow, par in jobs:
                csr = sbA.tile([1, 512], FP, tag="csr", name="csr", bufs=4)
                nc.vector.reciprocal_approx_fast(out=csr, in_=csrow)
                cs_d = dram.tile([1, 512], FP, tag="cs_d", name="cs_d", bufs=4)
                nc.sync.dma_start(out=cs_d[:], in_=csr)
                rec = sbA.tile([128, 512], FP, tag="rec", name="rec", bufs=4)
                recs = rec[par * 64:par * 64 + DH, :]
                cs_d_ap = cs_d.opt()
                nc.gpsimd.dma_start(out=recs, in_=bass.AP(
                    tensor=cs_d_ap.tensor, offset=cs_d_ap.offset,
                    ap=[[0, DH]] + list(cs_d_ap.ap[1:])))
                nc.vector.tensor_mul(out=dst, in0=dstu, in1=recs)

        def outproj_rs(s):
            sbB = pools["sbB"]
            for it4 in range(4):
                it = s * 4 + it4
                op_sb = sbB.tile([128, D], BF, tag="op", name="op_sb")
                for nh in range(2):
                    op_ps = psP.tile([128, 512], FP, tag="ps", name="op_ps")
                    for kt in range(2):
                        nc.tensor.matmul(op_ps, OT[kt][:, it * 128:(it + 1) * 128],
                                         wo_sb[:, kt, nh * 512:(nh + 1) * 512],
                                         start=(kt == 0), stop=(kt == 1))
                    nc.vector.tensor_copy(out=op_sb[:, nh * 512:(nh + 1) * 512],
                                          in_=op_ps)
                nc.sync.dma_start(
                    out=oph[s][:].rearrange("(t p) d -> t p d", p=128)[it4],
                    in_=op_sb)
            nc.gpsimd.collective_compute(
                "ReduceScatter", mybir.AluOpType.add,
                replica_groups=[[0, 1, 2, 3], [4, 5, 6, 7]],
                ins=[oph[s].opt()], outs=[rsh[s].opt()])

        def ln(s):
            sbB = pools["sbB"]
            xr = sbB.tile([128, D], FP, tag="xr", name="xr")
            rd = sbB.tile([128, D], FP, tag="rd", name="rd")
            rs_sb = sbB.tile([128, D], BF, tag="rsld", name="rs_sb")
            nc.sync.dma_start(out=rd, in_=res_tiles[s])
            nc.sync.dma_start(out=rs_sb, in_=rsh[s][:])
            nc.vector.tensor_add(out=xr, in0=rs_sb, in1=rd)
            stats = sbB.tile([128, 2, 6], FP, tag="stats", name="stats")
            mv = sbB.tile([128, 2], FP, tag="mv", name="mv")
            nc.vector.bn_stats(out=stats[:, 0, :], in_=xr[:, 0:512])
            nc.vector.bn_stats(out=stats[:, 1, :], in_=xr[:, 512:1024])
            nc.vector.bn_aggr(out=mv, in_=stats)
            nc.scalar.activation(out=mv[:, 1:2], in_=mv[:, 1:2], func=AF.Ln,
                                 bias=eps_sb, scale=1.0)
            nc.scalar.activation(out=mv[:, 1:2], in_=mv[:, 1:2], func=AF.Exp,
                                 scale=-0.5)
            nc.vector.tensor_scalar(out=xr, in0=xr,
                                    scalar1=mv[:, 0:1], scalar2=mv[:, 1:2],
                                    op0=mybir.AluOpType.subtract,
                                    op1=mybir.AluOpType.mult)
            nc.vector.tensor_mul(out=xr, in0=xr, in1=g_sb)
            nc.vector.tensor_add(out=xr, in0=xr, in1=b_sb)
            nc.sync.dma_start(out=out_tiles[s], in_=xr)

        for s in range(NS):
            jobs = attention(s, 0)
            if s == 0:
                proj_qkp(1)
            jobs += attention(s, 1)
            normalize(jobs)
            if s == 0:
                ph12_ctx.close()
                pools["sbB"] = ctx.enter_context(tc.tile_pool(name="sbB", bufs=2))
            outproj_rs(s)
            if s >= 1:
                ln(s - 1)
        ln(NS - 1)

    nc.compile()
    return nc


_NC = None
_last_in_maps = None


def kernel(**inputs) -> np.ndarray:
    global _NC, _last_in_maps
    if _NC is None:
        _NC = build()
    nc = _NC

    q_s = np.asarray(inputs["q_s"], np.float32)
    pos = np.asarray(inputs["pos_emb"], np.float32)
    Wq = np.asarray(inputs["Wq"], np.float32)
    Wk = np.asarray(inputs["Wk"], np.float32)
    Wv = np.asarray(inputs["Wv"], np.float32)
    Wo = np.asarray(inputs["Wo"], np.float32)
    bo = np.asarray(inputs["bo"], np.float32)
    ln_g = np.asarray(inputs["ln_g"], np.float32)
    ln_b = np.asarray(inputs["ln_b"], np.float32)

    in_maps = []
    for c in range(NCORES):
        b, g = divmod(c, GRP)
        cs = slice(g * C, (g + 1) * C)
        resid = np.concatenate(
            [q_s[b][512 * s + 128 * g: 512 * s + 128 * (g + 1)] for s in range(NS)],
            axis=0) + bo[None, :]
        bf = ml_dtypes.bfloat16
        in_maps.append({
            "xT": np.ascontiguousarray(q_s[b].T.astype(bf)),
            "posT": np.ascontiguousarray(pos[b][:, cs].T),
            "wq": np.ascontiguousarray(Wq[:, cs].astype(bf)),
            "wk": np.ascontiguousarray(Wk[:, cs].astype(bf)),
            "wv": np.ascontiguousarray(Wv[:, cs].astype(bf)),
            "wo": np.ascontiguousarray(Wo[cs, :].astype(bf)),
            "resid": np.ascontiguousarray(resid),
            "ln_g": ln_g,
            "ln_b": ln_b,
        })

    _last_in_maps = in_maps
    res = run_bass_kernel_spmd(nc, in_maps, list(range(NCORES)))
    out = np.empty((B, N, D), np.float32)
    for c in range(NCORES):
        b, g = divmod(c, GRP)
        o = res.results[c]["out"]
        for s in range(NS):
            out[b, 512 * s + 128 * g: 512 * s + 128 * (g + 1), :] = \
                o[128 * s:128 * (s + 1)]
    return out



# revision 7
# speedup vs baseline: 1.0076x; 1.0076x over previous
import contextlib
import os
import sys

os.environ.setdefault("MYCRO_LOCAL_CACHE", "1")
for _p in ("/opt/trn_rl_repo",):
    if os.path.isdir(_p) and _p not in sys.path:
        sys.path.append(_p)

import ml_dtypes
import numpy as np

import concourse.bass as bass
from concourse import bacc
import concourse.mybir as mybir
import concourse.tile as tile
from concourse.bass_utils import run_bass_kernel_spmd

FP = mybir.dt.float32
BF = mybir.dt.bfloat16
AF = mybir.ActivationFunctionType

B, N, D, H = 2, 2048, 1024, 16
NCORES = 8
GRP = 4
HPC = H // GRP
C = D // GRP
R = N // GRP
DH = D // H
SCALE = DH ** -0.5
LN_EPS = 1e-5

NT = N // 128
KD = D // 128
NS = N // 512

ATT_DT = BF
PROJ_DT = BF


def build():
    nc = bacc.Bacc("TRN2", target_bir_lowering=False, num_devices=NCORES)

    xT_t = nc.dram_tensor("xT", [D, N], PROJ_DT, kind="ExternalInput")
    posT_t = nc.dram_tensor("posT", [C, N], FP, kind="ExternalInput")
    wq_t = nc.dram_tensor("wq", [D, C], PROJ_DT, kind="ExternalInput")
    wk_t = nc.dram_tensor("wk", [D, C], PROJ_DT, kind="ExternalInput")
    wv_t = nc.dram_tensor("wv", [D, C], PROJ_DT, kind="ExternalInput")
    wo_t = nc.dram_tensor("wo", [C, D], BF, kind="ExternalInput")
    res_t = nc.dram_tensor("resid", [R, D], FP, kind="ExternalInput")
    g_t = nc.dram_tensor("ln_g", [D], FP, kind="ExternalInput")
    bt_t = nc.dram_tensor("ln_b", [D], FP, kind="ExternalInput")
    out_t = nc.dram_tensor("out", [R, D], FP, kind="ExternalOutput")

    res_tiles = res_t.ap().rearrange("(t p) d -> t p d", p=128)
    out_tiles = out_t.ap().rearrange("(t p) d -> t p d", p=128)

    def bcast_ap(ap, parts):
        return bass.AP(tensor=ap.tensor, offset=ap.offset,
                       ap=[[0, parts]] + list(ap.ap))

    with tile.TileContext(nc) as tc, contextlib.ExitStack() as ctx:
        persist = ctx.enter_context(tc.tile_pool(name="persist", bufs=1))
        attnp = ctx.enter_context(tc.tile_pool(name="attnp", bufs=1))
        psP = ctx.enter_context(tc.tile_pool(name="psP", bufs=2, space="PSUM"))
        psO = ctx.enter_context(tc.tile_pool(name="psO", bufs=2, space="PSUM"))
        psC = ctx.enter_context(tc.tile_pool(name="psC", bufs=2, space="PSUM"))
        dram = ctx.enter_context(tc.tile_pool(name="dram", bufs=1, space="DRAM"))

        ones64 = persist.tile([1, DH], FP, tag="ones64")
        nc.vector.memset(ones64, 1.0)
        onescol = persist.tile([128, 1], FP, tag="onescol")
        nc.vector.memset(onescol, 1.0)

        sbA = ctx.enter_context(tc.tile_pool(name="sbA", bufs=3))
        sbB = ctx.enter_context(tc.tile_pool(name="sbB", bufs=2))

        ph12_ctx = contextlib.ExitStack()
        p12 = ph12_ctx.enter_context(tc.tile_pool(name="ph12", bufs=1))

        wq_sb = p12.tile([128, KD, C], PROJ_DT, tag="wq")
        wk_sb = p12.tile([128, KD, C], PROJ_DT, tag="wk")
        wv_sb = p12.tile([128, KD, C], PROJ_DT, tag="wv")
        xT_sb = p12.tile([128, KD, N], PROJ_DT, tag="xT")
        posT_sb = p12.tile([128, 2, N], FP, tag="posT")
        xT_src = xT_t.ap().rearrange("(k p) n -> p k n", p=128)
        posT_src = posT_t.ap().rearrange("(m p) n -> p m n", p=128)

        nc.sync.dma_start(out=wk_sb, in_=wk_t.ap().rearrange("(k p) c -> p k c", p=128))
        nc.sync.dma_start(out=posT_sb[:, 0, :], in_=posT_src[:, 0, :])
        for k in range(KD):
            nc.sync.dma_start(out=xT_sb[:, k, 0:512], in_=xT_src[:, k, 0:512])
        nc.sync.dma_start(out=wq_sb, in_=wq_t.ap().rearrange("(k p) c -> p k c", p=128))
        nc.sync.dma_start(out=wv_sb, in_=wv_t.ap().rearrange("(k p) c -> p k c", p=128))
        nc.sync.dma_start(out=posT_sb[:, 1, :], in_=posT_src[:, 1, :])
        for s4 in range(1, 4):
            for k in range(KD):
                nc.sync.dma_start(out=xT_sb[:, k, s4 * 512:(s4 + 1) * 512],
                                  in_=xT_src[:, k, s4 * 512:(s4 + 1) * 512])
        xT = [xT_sb[:, k, :] for k in range(KD)]
        posT = [posT_sb[:, m, :] for m in range(2)]

        wo_sb = persist.tile([128, 2, D], BF, tag="wo")
        nc.sync.dma_start(out=wo_sb, in_=wo_t.ap().rearrange("(k p) d -> p k d", p=128))
        g_sb = persist.tile([128, D], FP, tag="g")
        b_sb = persist.tile([128, D], FP, tag="b")
        nc.gpsimd.dma_start(out=g_sb, in_=bcast_ap(g_t.ap(), 128))
        nc.gpsimd.dma_start(out=b_sb, in_=bcast_ap(bt_t.ap(), 128))
        eps_sb = persist.tile([128, 1], FP, tag="eps")
        nc.vector.memset(eps_sb, LN_EPS)
        res_sb = persist.tile([128, NS, D], FP, tag="res")
        for s in range(NS):
            nc.sync.dma_start(out=res_sb[:, s, :], in_=res_tiles[s])

        qT = [attnp.tile([128, N], ATT_DT, name=f"qT{m}", tag=f"qT{m}") for m in range(2)]
        kpT = [attnp.tile([128, N], ATT_DT, name=f"kpT{m}", tag=f"kpT{m}") for m in range(2)]
        V = [attnp.tile([128, HPC, DH + 1], ATT_DT, name=f"V{t}", tag=f"V{t}")
             for t in range(NT)]

        def proj_kp(m, s):
            kp_ps = psP.tile([128, 512], FP, tag="ps", name="kp_ps")
            for k in range(KD):
                nc.tensor.matmul(kp_ps, wk_sb[:, k, m * 128:(m + 1) * 128],
                                 xT[k][:, s * 512:(s + 1) * 512],
                                 start=(k == 0), stop=(k == KD - 1))
            nc.vector.tensor_add(out=kpT[m][:, s * 512:(s + 1) * 512],
                                 in0=kp_ps, in1=posT[m][:, s * 512:(s + 1) * 512])

        def proj_q(m, s):
            q_ps = psP.tile([128, 512], FP, tag="ps", name="q_ps")
            for k in range(KD):
                nc.tensor.matmul(q_ps, wq_sb[:, k, m * 128:(m + 1) * 128],
                                 xT[k][:, s * 512:(s + 1) * 512],
                                 start=(k == 0), stop=(k == KD - 1))
            nc.vector.tensor_copy(out=qT[m][:, s * 512:(s + 1) * 512], in_=q_ps)

        def proj_v(t):
            v_ps = psP.tile([128, C], FP, tag="ps", name="v_ps")
            for k in range(KD):
                nc.tensor.matmul(v_ps, xT[k][:, t * 128:(t + 1) * 128], wv_sb[:, k, :],
                                 start=(k == 0), stop=(k == KD - 1))
            nc.vector.tensor_copy(out=V[t][:, :, 0:DH],
                                  in_=v_ps.rearrange("p (h d) -> p h d", h=HPC))
            nc.vector.tensor_copy(out=V[t][:, :, DH:DH + 1],
                                  in_=onescol.broadcast_to([128, HPC, 1]))

        OT = [attnp.tile([128, N], BF, name=f"OT{m}", tag=f"OT{m}") for m in range(2)]
        OTU = [attnp.tile([128, N], FP, name=f"OTU{m}", tag=f"OTU{m}") for m in range(2)]
        oph = [[dram.tile([256, D], BF, name=f"oph{s}_{h}", tag=f"oph{s}_{h}")
                for h in range(2)] for s in range(NS)]
        rsh = [[dram.tile([64, D], BF, name=f"rsh{s}_{h}", tag=f"rsh{s}_{h}")
                for h in range(2)] for s in range(NS)]

        def attention(s, hp, fillers):
            ot_e = psO.tile([128, 512], FP, tag="ot", name="ot_e")
            ot_o = psO.tile([128, 512], FP, tag="ot", name="ot_o")
            stes = {}
            for jt in range(NT + 1):
                if jt < NT:
                    st = psC.tile([128, 1024], FP, tag="st", name="st")
                    nc.tensor.matmul(st[:, 0:512],
                                     kpT[hp][0:64, jt * 128:(jt + 1) * 128],
                                     qT[hp][0:64, s * 512:(s + 1) * 512],
                                     start=True, stop=True)
                    nc.tensor.matmul(st[:, 512:1024],
                                     kpT[hp][64:128, jt * 128:(jt + 1) * 128],
                                     qT[hp][64:128, s * 512:(s + 1) * 512],
                                     start=True, stop=True)
                    ste = sbA.tile([128, 1024], ATT_DT, tag="ste", name="ste")
                    nc.scalar.activation(out=ste, in_=st, func=AF.Exp, scale=SCALE)
                    stes[jt] = ste
                for f in fillers.get(jt, ()):
                    f()
                j2 = jt - 1
                if j2 >= 0:
                    ste2 = stes.pop(j2)
                    nc.tensor.matmul(ot_e[0:DH + 1, :], V[j2][:, 2 * hp, :],
                                     ste2[:, 0:512],
                                     start=(j2 == 0), stop=(j2 == NT - 1))
                    nc.tensor.matmul(ot_o[0:DH + 1, :], V[j2][:, 2 * hp + 1, :],
                                     ste2[:, 512:1024],
                                     start=(j2 == 0), stop=(j2 == NT - 1))
            jobs = []
            for par, ot in ((0, ot_e), (1, ot_o)):
                csrow = sbA.tile([1, 512], FP, tag="csrow", name="csrow", bufs=8)
                nc.vector.tensor_copy(out=csrow, in_=ot[DH:DH + 1, :])
                dst = OT[hp][par * 64:par * 64 + DH, s * 512:(s + 1) * 512]
                dstu = OTU[hp][par * 64:par * 64 + DH, s * 512:(s + 1) * 512]
                nc.vector.tensor_copy(out=dstu, in_=ot[0:DH, :])
                jobs.append((dst, dstu, csrow, par))
            return jobs

        def normalize(jobs):
            for i in range(0, len(jobs), 2):
                rec = psP.tile([128, 512], FP, tag="ps", name="rec")
                for dst, dstu, csrow, par in jobs[i:i + 2]:
                    csr = sbA.tile([1, 512], FP, tag="csr", name="csr", bufs=4)
                    nc.vector.reciprocal_approx_fast(out=csr, in_=csrow)
                    rows = rec[par * 64:par * 64 + DH, :]
                    nc.tensor.matmul(rows, ones64, csr, start=True, stop=True)
                    nc.vector.tensor_mul(out=dst, in0=dstu, in1=rows)

        def outproj_block(s, it4):
            it = s * 4 + it4
            op_sb = sbB.tile([128, D], BF, tag="op", name="op_sb")
            for nh in range(2):
                op_ps = psP.tile([128, 512], FP, tag="ps", name="op_ps")
                for kt in range(2):
                    nc.tensor.matmul(op_ps, OT[kt][:, it * 128:(it + 1) * 128],
                                     wo_sb[:, kt, nh * 512:(nh + 1) * 512],
                                     start=(kt == 0), stop=(kt == 1))
                nc.vector.tensor_copy(out=op_sb[:, nh * 512:(nh + 1) * 512],
                                      in_=op_ps)
            h, it2 = divmod(it4, 2)
            nc.sync.dma_start(
                out=oph[s][h][:].rearrange("(t p) d -> t p d", p=128)[it2],
                in_=op_sb)

        def rs_half(s, h):
            nc.gpsimd.collective_compute(
                "ReduceScatter", mybir.AluOpType.add,
                replica_groups=[[0, 1, 2, 3], [4, 5, 6, 7]],
                ins=[oph[s][h].opt()], outs=[rsh[s][h].opt()])

        def ln(s):
            xr = sbB.tile([128, D], FP, tag="xr", name="xr")
            rs_sb = sbB.tile([128, D], BF, tag="rsld", name="rs_sb")
            nc.sync.dma_start(out=rs_sb[0:64, :], in_=rsh[s][0][:])
            nc.sync.dma_start(out=rs_sb[64:128, :], in_=rsh[s][1][:])
            nc.vector.tensor_add(out=xr, in0=rs_sb, in1=res_sb[:, s, :])
            stats = sbB.tile([128, 2, 6], FP, tag="stats", name="stats")
            mv = sbB.tile([128, 2], FP, tag="mv", name="mv")
            nc.vector.bn_stats(out=stats[:, 0, :], in_=xr[:, 0:512])
            nc.vector.bn_stats(out=stats[:, 1, :], in_=xr[:, 512:1024])
            nc.vector.bn_aggr(out=mv, in_=stats)
            nc.scalar.activation(out=mv[:, 1:2], in_=mv[:, 1:2], func=AF.Ln,
                                 bias=eps_sb, scale=1.0)
            nc.scalar.activation(out=mv[:, 1:2], in_=mv[:, 1:2], func=AF.Exp,
                                 scale=-0.5)
            nc.vector.tensor_scalar(out=xr, in0=xr,
                                    scalar1=mv[:, 0:1], scalar2=mv[:, 1:2],
                                    op0=mybir.AluOpType.subtract,
                                    op1=mybir.AluOpType.mult)
            nc.vector.tensor_mul(out=xr, in0=xr, in1=g_sb)
            nc.vector.tensor_add(out=xr, in0=xr, in1=b_sb)
            nc.sync.dma_start(out=out_tiles[s], in_=xr)

        for s4 in range(NS):
            proj_kp(0, s4)
        for s4 in range(NS):
            proj_q(0, s4)
        proj_v(0)
        proj_v(1)

        jobs0 = attention(0, 0, {jt: (lambda t=jt + 2: proj_v(t),)
                                 for jt in range(NT - 2)})
        for s4 in range(NS):
            proj_kp(1, s4)
        for s4 in range(NS):
            proj_q(1, s4)
        jobs0 += attention(0, 1, {})
        pend_norm = {0: jobs0}

        for s in range(1, NS):
            f0 = {
                1: (lambda ss=s - 1: normalize(pend_norm.pop(ss)),),
                4: (lambda ss=s - 1: outproj_block(ss, 0),),
                7: (lambda ss=s - 1: outproj_block(ss, 1),
                    lambda ss=s - 1: rs_half(ss, 0)),
                10: (lambda ss=s - 1: outproj_block(ss, 2),),
                13: (lambda ss=s - 1: outproj_block(ss, 3),
                     lambda ss=s - 1: rs_half(ss, 1)),
            }
            jobs = attention(s, 0, f0)
            f1 = {}
            if s >= 2:
                f1[8] = (lambda ss=s - 2: ln(ss),)
            jobs += attention(s, 1, f1)
            pend_norm[s] = jobs

        normalize(pend_norm.pop(NS - 1))
        outproj_block(NS - 1, 0)
        outproj_block(NS - 1, 1)
        rs_half(NS - 1, 0)
        outproj_block(NS - 1, 2)
        outproj_block(NS - 1, 3)
        rs_half(NS - 1, 1)
        ln(NS - 2)
        ln(NS - 1)
        ph12_ctx.close()

    nc.compile()
    return nc


_NC = None
_last_in_maps = None


def kernel(**inputs) -> np.ndarray:
    global _NC, _last_in_maps
    if _NC is None:
        _NC = build()
    nc = _NC

    q_s = np.asarray(inputs["q_s"], np.float32)
    pos = np.asarray(inputs["pos_emb"], np.float32)
    Wq = np.asarray(inputs["Wq"], np.float32)
    Wk = np.asarray(inputs["Wk"], np.float32)
    Wv = np.asarray(inputs["Wv"], np.float32)
    Wo = np.asarray(inputs["Wo"], np.float32)
    bo = np.asarray(inputs["bo"], np.float32)
    ln_g = np.asarray(inputs["ln_g"], np.float32)
    ln_b = np.asarray(inputs["ln_b"], np.float32)

    in_maps = []
    for c in range(NCORES):
        b, g = divmod(c, GRP)
        cs = slice(g * C, (g + 1) * C)
        resid = np.concatenate(
            [q_s[b][512 * s + 256 * h + 64 * g: 512 * s + 256 * h + 64 * (g + 1)]
             for s in range(NS) for h in range(2)],
            axis=0) + bo[None, :]
        bf = ml_dtypes.bfloat16
        in_maps.append({
            "xT": np.ascontiguousarray(q_s[b].T.astype(bf)),
            "posT": np.ascontiguousarray(pos[b][:, cs].T),
            "wq": np.ascontiguousarray(Wq[:, cs].astype(bf)),
            "wk": np.ascontiguousarray(Wk[:, cs].astype(bf)),
            "wv": np.ascontiguousarray(Wv[:, cs].astype(bf)),
            "wo": np.ascontiguousarray(Wo[cs, :].astype(bf)),
            "resid": np.ascontiguousarray(resid),
            "ln_g": ln_g,
            "ln_b": ln_b,
        })

    _last_in_maps = in_maps
    res = run_bass_kernel_spmd(nc, in_maps, list(range(NCORES)))
    out = np.empty((B, N, D), np.float32)
    for c in range(NCORES):
        b, g = divmod(c, GRP)
        o = res.results[c]["out"]
        for s in range(NS):
            for h in range(2):
                out[b, 512 * s + 256 * h + 64 * g: 512 * s + 256 * h + 64 * (g + 1), :] = \
                    o[128 * s + 64 * h:128 * s + 64 * h + 64]
    return out


# revision 16
# speedup vs baseline: 1.0754x; 1.0673x over previous
import contextlib
import os
import sys

os.environ.setdefault("MYCRO_LOCAL_CACHE", "1")
for _p in ("/opt/trn_rl_repo",):
    if os.path.isdir(_p) and _p not in sys.path:
        sys.path.append(_p)

import ml_dtypes
import numpy as np

import concourse.bass as bass
from concourse import bacc
import concourse.mybir as mybir
import concourse.tile as tile
from concourse.bass_utils import run_bass_kernel_spmd

FP = mybir.dt.float32
BF = mybir.dt.bfloat16
F8 = mybir.dt.float8e4
DR = mybir.MatmulPerfMode.DoubleRow
AF = mybir.ActivationFunctionType

B, N, D, H = 2, 2048, 1024, 16
NCORES = 8
GRP = 4
HPC = H // GRP
C = D // GRP
R = N // GRP
DH = D // H
SCALE = DH ** -0.5
LN_EPS = 1e-5

NT = N // 128
KD = D // 128
NS = N // 512

ATT_DT = BF
PROJ_DT = F8
STE_DT = F8


def build():
    nc = bacc.Bacc("TRN2", target_bir_lowering=False, num_devices=NCORES)

    xT_t = nc.dram_tensor("xT", [D, N], PROJ_DT, kind="ExternalInput")
    posT_t = nc.dram_tensor("posT", [C, N], FP, kind="ExternalInput")
    wq_t = nc.dram_tensor("wq", [D, C], PROJ_DT, kind="ExternalInput")
    wk_t = nc.dram_tensor("wk", [D, C], PROJ_DT, kind="ExternalInput")
    wv_t = nc.dram_tensor("wv", [D, C], PROJ_DT, kind="ExternalInput")
    wo_t = nc.dram_tensor("wo", [C, D], BF, kind="ExternalInput")
    res_t = nc.dram_tensor("resid", [R, D], FP, kind="ExternalInput")
    g_t = nc.dram_tensor("ln_g", [D], FP, kind="ExternalInput")
    bt_t = nc.dram_tensor("ln_b", [D], FP, kind="ExternalInput")
    out_t = nc.dram_tensor("out", [R, D], FP, kind="ExternalOutput")

    res_tiles = res_t.ap().rearrange("(t p) d -> t p d", p=128)
    out_tiles = out_t.ap().rearrange("(t p) d -> t p d", p=128)

    def bcast_ap(ap, parts):
        return bass.AP(tensor=ap.tensor, offset=ap.offset,
                       ap=[[0, parts]] + list(ap.ap))

    with tile.TileContext(nc) as tc, contextlib.ExitStack() as ctx:
        persist = ctx.enter_context(tc.tile_pool(name="persist", bufs=1))
        attnp = ctx.enter_context(tc.tile_pool(name="attnp", bufs=1))
        psP = ctx.enter_context(tc.tile_pool(name="psP", bufs=2, space="PSUM"))
        psO = ctx.enter_context(tc.tile_pool(name="psO", bufs=2, space="PSUM"))
        psC = ctx.enter_context(tc.tile_pool(name="psC", bufs=2, space="PSUM"))
        dram = ctx.enter_context(tc.tile_pool(name="dram", bufs=1, space="DRAM"))

        ones64 = persist.tile([1, DH], FP, tag="ones64")
        nc.vector.memset(ones64, 1.0)
        onescol = persist.tile([128, 1], FP, tag="onescol")
        nc.vector.memset(onescol, 1.0)

        sbA = ctx.enter_context(tc.tile_pool(name="sbA", bufs=3))
        sbB = ctx.enter_context(tc.tile_pool(name="sbB", bufs=2))

        ph12_ctx = contextlib.ExitStack()
        p12 = ph12_ctx.enter_context(tc.tile_pool(name="ph12", bufs=1))

        wq_sb = p12.tile([128, KD, C], PROJ_DT, tag="wq")
        wk_sb = p12.tile([128, KD, C], PROJ_DT, tag="wk")
        wv_sb = p12.tile([128, KD, C], PROJ_DT, tag="wv")
        xT_sb = p12.tile([128, KD, N], PROJ_DT, tag="xT")
        posT_sb = p12.tile([128, 2, N], FP, tag="posT")
        xT_src = xT_t.ap().rearrange("(k p) n -> p k n", p=128)
        posT_src = posT_t.ap().rearrange("(m p) n -> p m n", p=128)

        nc.sync.dma_start(out=wk_sb, in_=wk_t.ap().rearrange("(k p) c -> p k c", p=128))
        nc.sync.dma_start(out=posT_sb[:, 0, :], in_=posT_src[:, 0, :])
        for k in range(KD):
            nc.sync.dma_start(out=xT_sb[:, k, 0:512], in_=xT_src[:, k, 0:512])
        nc.sync.dma_start(out=wq_sb, in_=wq_t.ap().rearrange("(k p) c -> p k c", p=128))
        nc.sync.dma_start(out=wv_sb, in_=wv_t.ap().rearrange("(k p) c -> p k c", p=128))
        nc.sync.dma_start(out=posT_sb[:, 1, :], in_=posT_src[:, 1, :])
        for s4 in range(1, 4):
            for k in range(KD):
                nc.sync.dma_start(out=xT_sb[:, k, s4 * 512:(s4 + 1) * 512],
                                  in_=xT_src[:, k, s4 * 512:(s4 + 1) * 512])
        xT = [xT_sb[:, k, :] for k in range(KD)]
        posT = [posT_sb[:, m, :] for m in range(2)]

        wo_sb = persist.tile([128, 2, D], BF, tag="wo")
        nc.sync.dma_start(out=wo_sb, in_=wo_t.ap().rearrange("(k p) d -> p k d", p=128))
        g_sb = persist.tile([128, D], FP, tag="g")
        b_sb = persist.tile([128, D], FP, tag="b")
        nc.gpsimd.dma_start(out=g_sb, in_=bcast_ap(g_t.ap(), 128))
        nc.gpsimd.dma_start(out=b_sb, in_=bcast_ap(bt_t.ap(), 128))
        eps_sb = persist.tile([128, 1], FP, tag="eps")
        nc.vector.memset(eps_sb, LN_EPS)
        shift_sb = persist.tile([128, 1], FP, tag="shift")
        nc.vector.memset(shift_sb, -4.0)
        res_sb = persist.tile([128, NS, D], FP, tag="res")
        for s in range(NS):
            nc.sync.dma_start(out=res_sb[:, s, :], in_=res_tiles[s])

        qT = [attnp.tile([128, N], ATT_DT, name=f"qT{m}", tag=f"qT{m}") for m in range(2)]
        kpT = [attnp.tile([128, N], ATT_DT, name=f"kpT{m}", tag=f"kpT{m}") for m in range(2)]
        VP = DH + 16
        V2 = [attnp.tile([128, 2, HPC, VP], STE_DT, name=f"V{t}", tag=f"V{t}")
              for t in range(NT // 2)]

        def proj_kp(m, s):
            kp_ps = psP.tile([128, 512], FP, tag="ps", name="kp_ps")
            for k2 in range(KD // 2):
                nc.tensor.matmul(kp_ps,
                                 wk_sb[:, 2 * k2:2 * k2 + 2, m * 128:(m + 1) * 128],
                                 xT_sb[:, 2 * k2:2 * k2 + 2, s * 512:(s + 1) * 512],
                                 start=(k2 == 0), stop=(k2 == KD // 2 - 1),
                                 perf_mode=DR)
            nc.vector.tensor_add(out=kpT[m][:, s * 512:(s + 1) * 512],
                                 in0=kp_ps, in1=posT[m][:, s * 512:(s + 1) * 512])

        def proj_q(m, s):
            q_ps = psP.tile([128, 512], FP, tag="ps", name="q_ps")
            for k2 in range(KD // 2):
                nc.tensor.matmul(q_ps,
                                 wq_sb[:, 2 * k2:2 * k2 + 2, m * 128:(m + 1) * 128],
                                 xT_sb[:, 2 * k2:2 * k2 + 2, s * 512:(s + 1) * 512],
                                 start=(k2 == 0), stop=(k2 == KD // 2 - 1),
                                 perf_mode=DR)
            nc.vector.tensor_copy(out=qT[m][:, s * 512:(s + 1) * 512], in_=q_ps)

        def proj_v(t):
            v_ps = psP.tile([128, C], FP, tag="ps", name="v_ps")
            for k2 in range(KD // 2):
                nc.tensor.matmul(v_ps,
                                 xT_sb[:, 2 * k2:2 * k2 + 2, t * 128:(t + 1) * 128],
                                 wv_sb[:, 2 * k2:2 * k2 + 2, :],
                                 start=(k2 == 0), stop=(k2 == KD // 2 - 1),
                                 perf_mode=DR)
            vt = V2[t // 2][:, t % 2]
            if t % 2 == 0:
                nc.vector.memset(V2[t // 2][:, :, :, DH + 1:VP], 0.0)
            nc.vector.tensor_copy(out=vt[:, :, 0:DH],
                                  in_=v_ps.rearrange("p (h d) -> p h d", h=HPC))
            nc.vector.tensor_copy(out=vt[:, :, DH:DH + 1],
                                  in_=onescol.broadcast_to([128, HPC, 1]))

        OT = [attnp.tile([128, N], BF, name=f"OT{m}", tag=f"OT{m}") for m in range(2)]
        OTU = [attnp.tile([128, N], FP, name=f"OTU{m}", tag=f"OTU{m}") for m in range(2)]
        oph = [[dram.tile([256, D], BF, name=f"oph{s}_{h}", tag=f"oph{s}_{h}")
                for h in range(2)] for s in range(NS)]
        rsh = [[dram.tile([64, D], BF, name=f"rsh{s}_{h}", tag=f"rsh{s}_{h}")
                for h in range(2)] for s in range(NS)]

        def attention(s, hp, fillers):
            ot_e = psO.tile([128, 512], FP, tag="ot", name="ot_e")
            ot_o = psO.tile([128, 512], FP, tag="ot", name="ot_o")
            stes = {}
            stepair = None
            for jt in range(NT + 1):
                if jt < NT:
                    st = psC.tile([128, 1024], FP, tag="st", name="st")
                    nc.tensor.matmul(st[:, 0:512],
                                     kpT[hp][0:64, jt * 128:(jt + 1) * 128],
                                     qT[hp][0:64, s * 512:(s + 1) * 512],
                                     start=True, stop=True)
                    nc.tensor.matmul(st[:, 512:1024],
                                     kpT[hp][64:128, jt * 128:(jt + 1) * 128],
                                     qT[hp][64:128, s * 512:(s + 1) * 512],
                                     start=True, stop=True)
                    if jt % 2 == 0:
                        stepair = sbA.tile([128, 2, 1024], STE_DT, tag="ste",
                                           name="ste")
                        stes[jt // 2] = stepair
                    nc.scalar.activation(out=stepair[:, jt % 2, :], in_=st,
                                         func=AF.Exp, scale=SCALE, bias=shift_sb)
                for f in fillers.get(jt, ()):
                    f()
                if jt >= 2 and jt % 2 == 0:
                    jj = (jt - 2) // 2
                    sp = stes.pop(jj)
                    nc.tensor.matmul(ot_e[0:DH + 1, :], V2[jj][:, :, 2 * hp, 0:DH + 1],
                                     sp[:, :, 0:512],
                                     start=(jj == 0), stop=(jj == NT // 2 - 1),
                                     perf_mode=DR)
                    nc.tensor.matmul(ot_o[0:DH + 1, :], V2[jj][:, :, 2 * hp + 1, 0:DH + 1],
                                     sp[:, :, 512:1024],
                                     start=(jj == 0), stop=(jj == NT // 2 - 1),
                                     perf_mode=DR)
            jobs = []
            for par, ot in ((0, ot_e), (1, ot_o)):
                csrow = sbA.tile([1, 512], FP, tag="csrow", name="csrow", bufs=8)
                nc.vector.tensor_copy(out=csrow, in_=ot[DH:DH + 1, :])
                dst = OT[hp][par * 64:par * 64 + DH, s * 512:(s + 1) * 512]
                dstu = OTU[hp][par * 64:par * 64 + DH, s * 512:(s + 1) * 512]
                nc.vector.tensor_copy(out=dstu, in_=ot[0:DH, :])
                jobs.append((dst, dstu, csrow, par))
            return jobs

        def normalize(jobs):
            for i in range(0, len(jobs), 2):
                rec = psP.tile([128, 512], FP, tag="ps", name="rec")
                for dst, dstu, csrow, par in jobs[i:i + 2]:
                    csr = sbA.tile([1, 512], FP, tag="csr", name="csr", bufs=4)
                    nc.vector.reciprocal_approx_fast(out=csr, in_=csrow)
                    rows = rec[par * 64:par * 64 + DH, :]
                    nc.tensor.matmul(rows, ones64, csr, start=True, stop=True)
                    nc.vector.tensor_mul(out=dst, in0=dstu, in1=rows)

        def outproj_block(s, it4):
            it = s * 4 + it4
            op_sb = sbB.tile([128, D], BF, tag="op", name="op_sb")
            for nh in range(2):
                op_ps = psP.tile([128, 512], FP, tag="ps", name="op_ps")
                for kt in range(2):
                    nc.tensor.matmul(op_ps, OT[kt][:, it * 128:(it + 1) * 128],
                                     wo_sb[:, kt, nh * 512:(nh + 1) * 512],
                                     start=(kt == 0), stop=(kt == 1))
                nc.vector.tensor_copy(out=op_sb[:, nh * 512:(nh + 1) * 512],
                                      in_=op_ps)
            h, it2 = divmod(it4, 2)
            nc.sync.dma_start(
                out=oph[s][h][:].rearrange("(t p) d -> t p d", p=128)[it2],
                in_=op_sb)

        def rs_half(s, h):
            nc.gpsimd.collective_compute(
                "ReduceScatter", mybir.AluOpType.add,
                replica_groups=[[0, 1, 2, 3], [4, 5, 6, 7]],
                ins=[oph[s][h].opt()], outs=[rsh[s][h].opt()])

        def ln(s):
            xr = sbB.tile([128, D], FP, tag="xr", name="xr")
            rs_sb = sbB.tile([128, D], BF, tag="rsld", name="rs_sb")
            nc.sync.dma_start(out=rs_sb[0:64, :], in_=rsh[s][0][:])
            nc.sync.dma_start(out=rs_sb[64:128, :], in_=rsh[s][1][:])
            nc.vector.tensor_add(out=xr, in0=rs_sb, in1=res_sb[:, s, :])
            stats = sbB.tile([128, 2, 6], FP, tag="stats", name="stats")
            mv = sbB.tile([128, 2], FP, tag="mv", name="mv")
            nc.vector.bn_stats(out=stats[:, 0, :], in_=xr[:, 0:512])
            nc.vector.bn_stats(out=stats[:, 1, :], in_=xr[:, 512:1024])
            nc.vector.bn_aggr(out=mv, in_=stats)
            nc.scalar.activation(out=mv[:, 1:2], in_=mv[:, 1:2], func=AF.Ln,
                                 bias=eps_sb, scale=1.0)
            nc.scalar.activation(out=mv[:, 1:2], in_=mv[:, 1:2], func=AF.Exp,
                                 scale=-0.5)
            nc.vector.tensor_scalar(out=xr, in0=xr,
                                    scalar1=mv[:, 0:1], scalar2=mv[:, 1:2],
                                    op0=mybir.AluOpType.subtract,
                                    op1=mybir.AluOpType.mult)
            nc.vector.tensor_mul(out=xr, in0=xr, in1=g_sb)
            nc.vector.tensor_add(out=xr, in0=xr, in1=b_sb)
            nc.sync.dma_start(out=out_tiles[s], in_=xr)

        for s4 in range(NS):
            proj_kp(0, s4)
        for s4 in range(NS):
            proj_q(0, s4)
        proj_v(0)
        proj_v(1)

        jobs0 = attention(0, 0, {jt: (lambda t=jt + 2: proj_v(t),)
                                 for jt in range(NT - 2)})
        for s4 in range(NS):
            proj_kp(1, s4)
        for s4 in range(NS):
            proj_q(1, s4)
        jobs0 += attention(0, 1, {})
        pend_norm = {0: jobs0}

        for s in range(1, NS):
            f0 = {
                1: (lambda ss=s - 1: normalize(pend_norm.pop(ss)),),
                4: (lambda ss=s - 1: outproj_block(ss, 0),),
                7: (lambda ss=s - 1: outproj_block(ss, 1),
                    lambda ss=s - 1: rs_half(ss, 0)),
                10: (lambda ss=s - 1: outproj_block(ss, 2),),
                13: (lambda ss=s - 1: outproj_block(ss, 3),
                     lambda ss=s - 1: rs_half(ss, 1)),
            }
            jobs = attention(s, 0, f0)
            f1 = {}
            if s >= 2:
                f1[8] = (lambda ss=s - 2: ln(ss),)
            jobs += attention(s, 1, f1)
            pend_norm[s] = jobs

        normalize(pend_norm.pop(NS - 1))
        outproj_block(NS - 1, 0)
        outproj_block(NS - 1, 1)
        rs_half(NS - 1, 0)
        outproj_block(NS - 1, 2)
        outproj_block(NS - 1, 3)
        rs_half(NS - 1, 1)
        ln(NS - 2)
        ln(NS - 1)
        ph12_ctx.close()

    nc.compile()
    return nc


_NC = None
_last_in_maps = None


def kernel(**inputs) -> np.ndarray:
    global _NC, _last_in_maps
    if _NC is None:
        _NC = build()
    nc = _NC

    q_s = np.asarray(inputs["q_s"], np.float32)
    pos = np.asarray(inputs["pos_emb"], np.float32)
    Wq = np.asarray(inputs["Wq"], np.float32)
    Wk = np.asarray(inputs["Wk"], np.float32)
    Wv = np.asarray(inputs["Wv"], np.float32)
    Wo = np.asarray(inputs["Wo"], np.float32)
    bo = np.asarray(inputs["bo"], np.float32)
    ln_g = np.asarray(inputs["ln_g"], np.float32)
    ln_b = np.asarray(inputs["ln_b"], np.float32)

    in_maps = []
    for c in range(NCORES):
        b, g = divmod(c, GRP)
        cs = slice(g * C, (g + 1) * C)
        resid = np.concatenate(
            [q_s[b][512 * s + 256 * h + 64 * g: 512 * s + 256 * h + 64 * (g + 1)]
             for s in range(NS) for h in range(2)],
            axis=0) + bo[None, :]
        bf = ml_dtypes.bfloat16
        f8 = ml_dtypes.float8_e4m3
        in_maps.append({
            "xT": np.ascontiguousarray(q_s[b].T.astype(f8)),
            "posT": np.ascontiguousarray(pos[b][:, cs].T),
            "wq": np.ascontiguousarray(Wq[:, cs].astype(f8)),
            "wk": np.ascontiguousarray(Wk[:, cs].astype(f8)),
            "wv": np.ascontiguousarray(Wv[:, cs].astype(f8)),
            "wo": np.ascontiguousarray(Wo[cs, :].astype(bf)),
            "resid": np.ascontiguousarray(resid),
            "ln_g": ln_g,
            "ln_b": ln_b,
        })

    _last_in_maps = in_maps
    res = run_bass_kernel_spmd(nc, in_maps, list(range(NCORES)))
    out = np.empty((B, N, D), np.float32)
    for c in range(NCORES):
        b, g = divmod(c, GRP)
        o = res.results[c]["out"]
        for s in range(NS):
            for h in range(2):
                out[b, 512 * s + 256 * h + 64 * g: 512 * s + 256 * h + 64 * (g + 1), :] = \
                    o[128 * s + 64 * h:128 * s + 64 * h + 64]
    return out
